# revision 1
# baseline (speedup 1.0000x reference)
"""Causal self-attention Trainium2 kernel.

Sharding: 8 cores = (4 batches) x (2 head-groups of 8 heads).
Each core: projections for its 512 channels, causal attention for its 8
heads over its batch, partial out-projection over its 512 channels.
Host: sums the two partials per batch and adds the output bias.

All matmul operands are bf16 (accumulation in fp32 PSUM); softmax,
normalization and the output partials stay fp32.

Layouts on core (b = fixed batch, channels o in [0,512) local):
  xT   [128f, 8fc, 2048t] bf16 - loaded via DMA transpose from DRAM
  qT/kT [128o, 4oc, 2048t] bf16 - head h = oc*2+hh on partitions hh*64..+64 of chunk oc
  vx   [128t, 16tj, 8h*65] bf16 - v natural + ones column per head (softmax denom)
  scores^T tiles [128j, 512i] f32 psum -> exp on ACT (scale=1/8) -> bf16, masked
  attn^T accum psum [65, 512i] f32: rows 0..63 head out, row 64 denom
  out  psum [128t, 512c] f32 -> sbuf -> DRAM partial
"""

from contextlib import ExitStack

import ml_dtypes
import numpy as np

import concourse.bass as bass
import concourse.mybir as mybir
import concourse.tile as tile

P = 128
C = 1024  # d_model
CL = 512  # local channels (8 heads * 64)
D = 64  # head dim
NH = 8  # local heads
FC = C // P  # 8 f-chunks
OC = CL // P  # 4 o-chunks
F32 = mybir.dt.float32
F32R = mybir.dt.float32r
BF16 = mybir.dt.bfloat16
AF = mybir.ActivationFunctionType
GROUP = 3  # score jt-tiles per exp call (3 psum banks, double buffered)


def _emit(nc, tc, ctx, T):
    NT = T // P  # 128-token chunks
    T4 = T // 512  # 512-token chunks

    xb = nc.dram_tensor("xb", [T, C], BF16, kind="ExternalInput")
    wq_d = nc.dram_tensor("wq", [C, CL], BF16, kind="ExternalInput")
    wk_d = nc.dram_tensor("wk", [C, CL], BF16, kind="ExternalInput")
    wv_d = nc.dram_tensor("wv", [C, CL], BF16, kind="ExternalInput")
    wo_d = nc.dram_tensor("wo", [CL, C], BF16, kind="ExternalInput")
    bq_d = nc.dram_tensor("bq", [CL], F32, kind="ExternalInput")
    bk_d = nc.dram_tensor("bk", [CL], F32, kind="ExternalInput")
    bv_d = nc.dram_tensor("bv", [CL], BF16, kind="ExternalInput")
    stair_d = nc.dram_tensor("stair", [P, 1024], BF16, kind="ExternalInput")
    outp = nc.dram_tensor("outp", [T, C], F32, kind="ExternalOutput")

    const = ctx.enter_context(tc.tile_pool(name="const", bufs=1))
    ones1 = const.tile([1, P], BF16)
    nc.gpsimd.memset(ones1[:], 1.0)

    bq_sb = const.tile([P, OC], F32)
    nc.sync.dma_start(bq_sb[:], bq_d.rearrange("(oc p) -> p oc", p=P))
    bk_sb = const.tile([P, OC], F32)
    nc.sync.dma_start(bk_sb[:], bk_d.rearrange("(oc p) -> p oc", p=P))
    bv_sb = const.tile([1, CL], BF16)
    nc.sync.dma_start(bv_sb[:], bv_d.rearrange("(a c) -> a c", a=1))
    stair_sb = const.tile([P, 1024], BF16)
    nc.sync.dma_start(stair_sb[:], stair_d[:])

    qkv = ctx.enter_context(tc.tile_pool(name="qkv", bufs=1))
    qT = qkv.tile([P, OC, T], BF16)
    kT = qkv.tile([P, OC, T], BF16)
    vx = qkv.tile([P, NT, NH * 65], BF16)
    vx5 = vx.rearrange("p n (h u) -> p n h u", u=65)
    nc.gpsimd.memset(vx5[:, :, :, 64:65], 1.0)

    # ---------------- projections ----------------
    with (
        tc.tile_pool(name="wpool", bufs=1) as wpool,
        tc.tile_pool(name="xT_pool", bufs=1) as xT_pool,
        tc.tile_pool(name="pj_ps", bufs=8, space="PSUM") as pj_ps,
    ):
        xT = xT_pool.tile([P, FC, T], BF16)
        xbr = xb.rearrange("t (fc p) -> t fc p", p=P)
        for fc in range(FC):
            nc.sync.dma_start(xT[:, fc, :], xbr[:, fc, :], transpose=True)
        wq_sb = wpool.tile([P, FC, CL], BF16)
        nc.sync.dma_start(wq_sb[:], wq_d.rearrange("(fc p) o -> p fc o", p=P))
        wk_sb = wpool.tile([P, FC, CL], BF16)
        nc.sync.dma_start(wk_sb[:], wk_d.rearrange("(fc p) o -> p fc o", p=P))
        wv_sb = wpool.tile([P, FC, CL], BF16)
        nc.sync.dma_start(wv_sb[:], wv_d.rearrange("(fc p) o -> p fc o", p=P))

        for oc in range(OC):
            for w_sb, b_sb, dT in ((wq_sb, bq_sb, qT), (wk_sb, bk_sb, kT)):
                pss = [
                    pj_ps.tile([P, 512], F32, tag="pj", name=f"pj{oc}_{tt}")
                    for tt in range(T4)
                ]
                for fc in range(FC):
                    for tt in range(T4):
                        nc.tensor.matmul(
                            pss[tt][:],
                            w_sb[:, fc, oc * P : (oc + 1) * P],
                            xT[:, fc, tt * 512 : (tt + 1) * 512],
                            start=(fc == 0),
                            stop=(fc == FC - 1),
                        )
                for tt in range(T4):
                    nc.vector.tensor_scalar_add(
                        dT[:, oc, tt * 512 : (tt + 1) * 512],
                        pss[tt][:],
                        b_sb[:, oc : oc + 1],
                    )
        for s in range(NT):
            ps = pj_ps.tile([P, 512], F32, tag="pj", name=f"pjv{s}")
            for fc in range(FC):
                nc.tensor.matmul(
                    ps[:],
                    xT[:, fc, s * P : (s + 1) * P],
                    wv_sb[:, fc, :],
                    start=(fc == 0),
                    stop=False,
                )
            nc.tensor.matmul(
                ps[:],
                ones1[:],
                bv_sb[:],
                start=False,
                stop=True,
            )
            nc.vector.tensor_copy(
                vx5[:, s, :, 0:64],
                ps[:].rearrange("p (h d) -> p h d", d=D),
            )

    # ---------------- attention ----------------
    wo_pool = ctx.enter_context(tc.tile_pool(name="wo_pool", bufs=1))
    attT_pool = ctx.enter_context(tc.tile_pool(name="attT_pool", bufs=1))
    wo_sb = wo_pool.tile([P, OC, C], BF16)
    nc.sync.dma_start(wo_sb[:], wo_d.rearrange("(oc p) c -> p oc c", p=P))
    attT = attT_pool.tile([P, OC, T], BF16)

    with (
        tc.tile_pool(name="exp_pool", bufs=3) as exp_pool,
        tc.tile_pool(name="nrm", bufs=2) as nrm_pool,
        tc.tile_pool(name="sc_ps", bufs=2, space="PSUM") as sc_ps_pool,
        tc.tile_pool(name="at_ps", bufs=2, space="PSUM") as at_ps_pool,
    ):
        for oc in range(OC):
            for hh in range(2):
                h = oc * 2 + hh
                base = hh * 64
                for ic in range(T4):
                    njt = ic * 4 + 4
                    at = at_ps_pool.tile([P, 512], F32)
                    for g0 in range(0, njt, GROUP):
                        grp = list(range(g0, min(g0 + GROUP, njt)))
                        n = len(grp)
                        sc = sc_ps_pool.tile([P, GROUP, 512], F32)
                        for si, jt in enumerate(grp):
                            nc.tensor.matmul(
                                sc[:, si, :],
                                kT[base : base + D, oc, jt * P : (jt + 1) * P],
                                qT[base : base + D, oc, ic * 512 : (ic + 1) * 512],
                                start=True,
                                stop=True,
                            )
                        ex = exp_pool.tile([P, GROUP, 512], BF16)
                        nc.scalar.activation(
                            ex[:, 0:n, :], sc[:, 0:n, :], AF.Exp, scale=0.125
                        )
                        for si, jt in enumerate(grp):
                            d = jt - ic * 4
                            if d >= 0:
                                w = (d + 1) * P
                                nc.vector.tensor_mul(
                                    ex[:, si, 0:w],
                                    ex[:, si, 0:w],
                                    stair_sb[:, 512 - d * P : 512 - d * P + w],
                                )
                        for si, jt in enumerate(grp):
                            nc.tensor.matmul(
                                at[0:65, :],
                                vx5[:, jt, h, :],
                                ex[:, si, :],
                                start=(jt == 0),
                                stop=(jt == njt - 1),
                            )
                    rc = nrm_pool.tile([1, 512], BF16, tag="rc")
                    with nc.allow_low_precision(reason="softmax recip bcast"):
                        nc.vector.reciprocal(rc[:], at[64:65, :])
                    # recip row broadcast into psum rows 64..127 (K=1 outer)
                    nc.tensor.matmul(
                        at[64:128, :], ones1[:, 0:64], rc[:], start=True, stop=True
                    )
                    tmp = nrm_pool.tile([64, 512], F32, tag="tmp")
                    nc.vector.tensor_copy(tmp[:], at[0:64, :])
                    nc.vector.tensor_mul(
                        attT[base : base + D, oc, ic * 512 : (ic + 1) * 512],
                        tmp[:],
                        at[64:128, :],
                    )

    # ---------------- out-projection ----------------
    with (
        tc.tile_pool(name="op_ps", bufs=4, space="PSUM") as op_ps,
        tc.tile_pool(name="ob_pool", bufs=4) as ob_pool,
    ):
        for s16 in range(NT):
            for ch in range(2):
                ps = op_ps.tile([P, 512], F32)
                for oc in range(OC):
                    nc.tensor.matmul(
                        ps[:],
                        attT[:, oc, s16 * P : (s16 + 1) * P],
                        wo_sb[:, oc, ch * 512 : (ch + 1) * 512],
                        start=(oc == 0),
                        stop=(oc == OC - 1),
                    )
                ob = ob_pool.tile([P, 512], F32)
                nc.scalar.copy(ob[:], ps[:])
                nc.sync.dma_start(
                    outp[s16 * P : (s16 + 1) * P, ch * 512 : (ch + 1) * 512],
                    ob[:],
                )


def build(T=2048):
    nc = bass.Bass()
    with tile.TileContext(nc) as tc:
        with ExitStack() as ctx:
            _emit(nc, tc, ctx, T)
    return nc


def make_stair():
    j = np.arange(P)[:, None]
    u = np.arange(1024)[None, :]
    return (u >= j + 512).astype(ml_dtypes.bfloat16)


def make_in_maps(x, wq, bq, wk, bk, wv, bv, wo):
    bf = ml_dtypes.bfloat16
    stair = make_stair()
    in_maps = []
    for c in range(8):
        b, g = c // 2, c % 2
        sl = slice(g * CL, (g + 1) * CL)
        in_maps.append(
            {
                "xb": np.ascontiguousarray(x[b]).astype(bf),
                "wq": np.ascontiguousarray(wq[:, sl]).astype(bf),
                "wk": np.ascontiguousarray(wk[:, sl]).astype(bf),
                "wv": np.ascontiguousarray(wv[:, sl]).astype(bf),
                "wo": np.ascontiguousarray(wo[sl, :]).astype(bf),
                "bq": np.ascontiguousarray(bq[sl]),
                "bk": np.ascontiguousarray(bk[sl]),
                "bv": np.ascontiguousarray(bv[sl]).astype(bf),
                "stair": stair,
            }
        )
    return in_maps


_cache = {}


def _split_multi_waits(bir_json: bytes) -> bytes:
    """Split instructions carrying >1 sync waits into single-wait NoOp
    chains on the same engine queue.  The TPB instruction encoding has one
    wait slot; this walrus build refuses multi-wait instructions instead
    of splitting them itself."""
    import orjson

    m = orjson.loads(bir_json)
    n = 0
    for fn in m.get("functions", []):
        for blk in fn.get("blocks", []):
            out = []
            for inst in blk.get("instructions", []):
                si = inst.get("sync_info")
                waits = si.get("on_wait") if si else None
                if waits and len(waits) > 1:
                    for w in waits[:-1]:
                        n += 1
                        out.append(
                            {
                                "debug": inst.get("debug", {}),
                                "engine": inst["engine"],
                                "ins": [],
                                "outs": [],
                                "name": f"{inst['name']}_sw{n}",
                                "opcode": "NoOp",
                                "text_hint": "split_wait",
                                "sync_info": {"on_wait": [w], "on_update": []},
                            }
                        )
                    si["on_wait"] = [waits[-1]]
                out.append(inst)
            blk["instructions"] = out
    return orjson.dumps(m)


def _install_compile_patch():
    import concourse.bass_utils as bu

    if getattr(bu, "_split_waits_patched", False):
        return
    orig = bu.compile_bir_kernel

    def patched(bir_json, tmpdir, neff_name="file.neff"):
        return orig(_split_multi_waits(bir_json), tmpdir, neff_name)

    bu.compile_bir_kernel = patched
    bu._split_waits_patched = True
    try:
        import concourse.bass2jax as b2j

        b2j.compile_bir_kernel = patched
    except ImportError:
        pass


def kernel(x, wq, bq, wk, bk, wv, bv, wo, bo):
    from concourse.bass_utils import run_bass_kernel_spmd

    _install_compile_patch()

    x = np.asarray(x, np.float32)
    args = [np.asarray(a, np.float32) for a in (wq, bq, wk, bk, wv, bv, wo, bo)]
    wq, bq, wk, bk, wv, bv, wo, bo = args
    B, T, _ = x.shape

    if "nc" not in _cache:
        _cache["nc"] = build(T)
    nc = _cache["nc"]

    in_maps = make_in_maps(x, wq, bq, wk, bk, wv, bv, wo)
    res = run_bass_kernel_spmd(nc, in_maps, core_ids=list(range(8)))
    out = np.empty((B, T, C), np.float32)
    for b in range(B):
        out[b] = res.results[2 * b]["outp"] + res.results[2 * b + 1]["outp"] + bo
    return out



# revision 5
# speedup vs baseline: 1.3269x; 1.3269x over previous
"""Causal self-attention Trainium2 kernel.

Sharding: 8 cores = (4 batches) x (2 head-groups of 8 heads).
Each core: projections for its 512 channels, causal attention for its 8
heads over its batch, partial out-projection over its 512 channels.
Host: sums the two partials per batch and adds the output bias.

v2 design notes:
- q/k projections run as fp8 DoubleRow matmuls (2x128 contraction per
  pass -> half the PE row count); host pre-scales wq/wk by 32, the
  psum->sbuf conversion multiplies by 1/32 and adds the bias (on ACT,
  which idles during the projection phase). v stays bf16 (fp8 v noise
  breaks the 2e-2 budget).
- x arrives host-transposed (xT bf16 + xT8 fp8), no transpose DMAs.
- vx carries 64 ones-columns per head, so the attnV matmul broadcasts
  the softmax denominator onto psum partitions 64..127 for free
  (matmul cost depends only on the moving free size).
- normalization: DVE reciprocal [64,512] psum->bf16, then one
  tensor_mul (one-psum-operand rule) -> attT. No PE broadcast.
- emission is software-pipelined: scores of head-slot i interleave
  with attnV of slot i-1, and the out-projection of token-block ic
  interleaves into the head slots of ic+1, keeping PE fed while ACT
  works through the exps.
"""

from contextlib import ExitStack

import ml_dtypes
import numpy as np

import concourse.bass as bass
import concourse.mybir as mybir
import concourse.tile as tile

P = 128
C = 1024  # d_model
CL = 512  # local channels (8 heads * 64)
D = 64  # head dim
NH = 8  # local heads
FC = C // P  # 8 f-chunks
OC = CL // P  # 4 o-chunks
F32 = mybir.dt.float32
BF16 = mybir.dt.bfloat16
F8 = mybir.dt.float8e4
AF = mybir.ActivationFunctionType
DR = mybir.MatmulPerfMode.DoubleRow
GROUP = 3  # score jt-tiles per exp call


def _emit(nc, tc, ctx, T):
    NT = T // P  # 16 token chunks
    T4 = T // 512  # 4 ic-blocks

    xTd = nc.dram_tensor("xT", [C, T], BF16, kind="ExternalInput")
    xT8d = nc.dram_tensor("xT8", [C, T], F8, kind="ExternalInput")
    wq8d = nc.dram_tensor("wq8", [C, CL], F8, kind="ExternalInput")
    wk8d = nc.dram_tensor("wk8", [C, CL], F8, kind="ExternalInput")
    wvd = nc.dram_tensor("wv", [C, CL], BF16, kind="ExternalInput")
    wod = nc.dram_tensor("wo", [CL, C], BF16, kind="ExternalInput")
    bqd = nc.dram_tensor("bq", [CL], F32, kind="ExternalInput")
    bkd = nc.dram_tensor("bk", [CL], F32, kind="ExternalInput")
    bvd = nc.dram_tensor("bv", [CL], BF16, kind="ExternalInput")
    staird = nc.dram_tensor("stair", [P, 1024], BF16, kind="ExternalInput")
    outp = nc.dram_tensor("outp", [T, C], F32, kind="ExternalOutput")

    const = ctx.enter_context(tc.tile_pool(name="const", bufs=1))
    ones1 = const.tile([1, P], BF16)
    nc.gpsimd.memset(ones1[:], 1.0)
    bq_sb = const.tile([P, OC], F32)
    nc.sync.dma_start(bq_sb[:], bqd.rearrange("(oc p) -> p oc", p=P))
    bk_sb = const.tile([P, OC], F32)
    nc.sync.dma_start(bk_sb[:], bkd.rearrange("(oc p) -> p oc", p=P))
    bv_sb = const.tile([1, CL], BF16)
    nc.sync.dma_start(bv_sb[:], bvd.rearrange("(a c) -> a c", a=1))
    stair_sb = const.tile([P, 1024], BF16)
    nc.sync.dma_start(stair_sb[:], staird[:])

    main = ctx.enter_context(tc.tile_pool(name="main", bufs=1))
    qT = main.tile([P, OC, T], BF16)
    kT = main.tile([P, OC, T], BF16)
    vx = main.tile([P, NT, NH, P], BF16)
    nc.gpsimd.memset(vx[:, :, :, D:P], 1.0)
    attT = main.tile([P, OC, T], BF16)
    wo_sb = main.tile([P, OC, C], BF16)
    nc.sync.dma_start(wo_sb[:], wod.rearrange("(oc p) c -> p oc c", p=P))

    # ---------------- projections ----------------
    with (
        tc.tile_pool(name="wpool", bufs=1) as wpool,
        tc.tile_pool(name="xpool", bufs=1) as xpool,
        tc.tile_pool(name="pj_ps", bufs=4, space="PSUM") as pj_ps,
    ):
        xT8 = xpool.tile([P, FC, T], F8)
        xT8r = xT8d.rearrange("(fc p) t -> p fc t", p=P)
        for fc in range(FC):
            nc.sync.dma_start(xT8[:, fc, :], xT8r[:, fc, :])
        wq8_sb = wpool.tile([P, FC, CL], F8)
        nc.sync.dma_start(wq8_sb[:], wq8d.rearrange("(fc p) o -> p fc o", p=P))
        wk8_sb = wpool.tile([P, FC, CL], F8)
        nc.sync.dma_start(wk8_sb[:], wk8d.rearrange("(fc p) o -> p fc o", p=P))
        xT = xpool.tile([P, FC, T], BF16)
        xTr = xTd.rearrange("(fc p) t -> p fc t", p=P)
        for fc in range(FC):
            nc.sync.dma_start(xT[:, fc, :], xTr[:, fc, :])
        wv_sb = wpool.tile([P, FC, CL], BF16)
        nc.sync.dma_start(wv_sb[:], wvd.rearrange("(fc p) o -> p fc o", p=P))

        # q/k projections: fp8 DoubleRow, contraction 4 passes of 2x128
        for oc in range(OC):
            for w8, b_sb, dT in ((wq8_sb, bq_sb, qT), (wk8_sb, bk_sb, kT)):
                for tt in range(T4):
                    ps = pj_ps.tile([P, 512], F32, tag="pj", name=f"pj{oc}_{tt}_{dT is qT}")
                    for g in range(4):
                        nc.tensor.matmul(
                            ps[:],
                            w8[:, 2 * g : 2 * g + 2, oc * P : (oc + 1) * P],
                            xT8[:, 2 * g : 2 * g + 2, tt * 512 : (tt + 1) * 512],
                            start=(g == 0),
                            stop=(g == 3),
                            perf_mode=DR,
                        )
                    nc.scalar.activation(
                        dT[:, oc, tt * 512 : (tt + 1) * 512],
                        ps[:],
                        AF.Identity,
                        bias=b_sb[:, oc : oc + 1],
                        scale=0.03125,
                    )
        # v projection: bf16, xT chunks stationary, wv moving
        for s in range(NT):
            ps = pj_ps.tile([P, 512], F32, tag="pj", name=f"pjv{s}")
            for fc in range(FC):
                nc.tensor.matmul(
                    ps[:],
                    xT[:, fc, s * P : (s + 1) * P],
                    wv_sb[:, fc, :],
                    start=(fc == 0),
                    stop=False,
                )
            nc.tensor.matmul(ps[:], ones1[:], bv_sb[:], start=False, stop=True)
            nc.scalar.copy(
                vx[:, s, :, 0:D],
                ps[:].rearrange("p (h d) -> p h d", d=D),
            )

    # ---------------- attention + out-projection ----------------
    with (
        tc.tile_pool(name="ex_pool", bufs=7) as ex_pool,
        tc.tile_pool(name="nrm", bufs=2) as nrm_pool,
        tc.tile_pool(name="ob_pool", bufs=4) as ob_pool,
        tc.tile_pool(name="sc_ps", bufs=2, space="PSUM") as sc_ps_pool,
        tc.tile_pool(name="at_ps", bufs=2, space="PSUM") as at_ps_pool,
    ):
        # slot list: (ic, h); slot i's scores interleave with slot i-1's attnV
        slots = [(ic, h) for ic in range(T4) for h in range(NH)]

        def sc_emitters(ic, h, sc_tiles):
            """Score-group closures for (ic, h): matmul njt k-tiles in
            GROUP-sized chunks + exp on ACT; fill sc_tiles with
            (sc, ex, grp) for the attnV closures."""
            oc, hh = h // 2, h % 2
            base = hh * D
            njt = ic * 4 + 4
            outs = []
            for g0 in range(0, njt, GROUP):
                grp = list(range(g0, min(g0 + GROUP, njt)))

                def emit(grp=grp, oc=oc, base=base, ic=ic, h=h):
                    n = len(grp)
                    sc = sc_ps_pool.tile(
                        [P, GROUP, 512], F32, tag="sc", name=f"sc{ic}_{h}_{grp[0]}"
                    )
                    for si, jt in enumerate(grp):
                        nc.tensor.matmul(
                            sc[:, si, :],
                            kT[base : base + D, oc, jt * P : (jt + 1) * P],
                            qT[base : base + D, oc, ic * 512 : (ic + 1) * 512],
                            start=True,
                            stop=True,
                        )
                    ex = ex_pool.tile(
                        [P, GROUP, 512], BF16, tag="ex", name=f"ex{ic}_{h}_{grp[0]}"
                    )
                    nc.scalar.activation(
                        ex[:, 0:n, :], sc[:, 0:n, :], AF.Exp, scale=0.125
                    )
                    sc_tiles.append((ex, grp))

                outs.append(emit)
            return outs

        def att_emitters(ic, h, sc_tiles, at_box):
            """attnV-group closures for (ic, h): stair-mask diag tiles on
            DVE, then accumulate into the at psum tile."""
            njt = ic * 4 + 4
            outs = []
            ngrp = (njt + GROUP - 1) // GROUP

            for gi in range(ngrp):

                def emit(gi=gi, ic=ic, h=h):
                    ex, grp = sc_tiles[gi]
                    for si, jt in enumerate(grp):
                        d = jt - ic * 4
                        if d >= 0:
                            w = (d + 1) * P
                            nc.vector.tensor_mul(
                                ex[:, si, 0:w],
                                ex[:, si, 0:w],
                                stair_sb[:, 512 - d * P : 512 - d * P + w],
                            )
                    if gi == 0:
                        at_box.append(
                            at_ps_pool.tile([P, 512], F32, tag="at", name=f"at{ic}_{h}")
                        )
                    at = at_box[0]
                    njt_l = ic * 4 + 4
                    for si, jt in enumerate(grp):
                        nc.tensor.matmul(
                            at[:],
                            vx[:, jt, h, :],
                            ex[:, si, :],
                            start=(jt == 0),
                            stop=(jt == njt_l - 1),
                        )

                outs.append(emit)
            return outs

        def norm_emit(ic, h, at_box):
            oc, hh = h // 2, h % 2
            base = hh * D
            at = at_box[0]
            rc = nrm_pool.tile([D, 512], BF16, tag="rc", name=f"rc{ic}_{h}")
            with nc.allow_low_precision(reason="softmax recip bcast"):
                nc.vector.reciprocal(rc[:], at[D:P, :])
            nc.vector.tensor_mul(
                attT[base : base + D, oc, ic * 512 : (ic + 1) * 512],
                at[0:D, :],
                rc[:],
            )

        def op_emitters(ic):
            """Out-projection chains for token block ic: 4 token chunks x
            2 channel halves, each a 4-matmul chain + copy + store."""
            outs = []
            for s in range(4):
                for ch in range(2):

                    def emit(s=s, ch=ch, ic=ic):
                        s16 = ic * 4 + s
                        ps = at_ps_pool.tile(
                            [P, 512], F32, tag="at", name=f"op{ic}_{s}_{ch}"
                        )
                        for oc in range(OC):
                            nc.tensor.matmul(
                                ps[:],
                                attT[:, oc, s16 * P : (s16 + 1) * P],
                                wo_sb[:, oc, ch * 512 : (ch + 1) * 512],
                                start=(oc == 0),
                                stop=(oc == OC - 1),
                            )
                        ob = ob_pool.tile([P, 512], F32, tag="ob", name=f"ob{ic}_{s}_{ch}")
                        nc.vector.tensor_copy(ob[:], ps[:])
                        nc.gpsimd.dma_start(
                            outp[s16 * P : (s16 + 1) * P, ch * 512 : (ch + 1) * 512],
                            ob[:],
                        )

                    outs.append(emit)
            return outs

        prev = None  # (att_closures, norm_closure) of slot i-1
        pending_ops = []
        for ic, h in slots:
            sc_tiles = []
            at_box = []
            scs = sc_emitters(ic, h, sc_tiles)
            atts = att_emitters(ic, h, sc_tiles, at_box)
            norm = lambda ic=ic, h=h, ab=at_box: norm_emit(ic, h, ab)

            # interleave: this slot's scores with previous slot's attnV
            if prev is None:
                for e in scs:
                    e()
            else:
                patts, pnorm = prev
                # lead with two score groups, then alternate
                na, nb = len(scs), len(patts)
                ai = bi = 0
                while ai < na or bi < nb:
                    if ai < na:
                        e = scs[ai]
                        ai += 1
                        e()
                    if ai < na and ai <= bi + 1:
                        e = scs[ai]
                        ai += 1
                        e()
                    if bi < nb:
                        e = patts[bi]
                        bi += 1
                        e()
                pnorm()
                if pending_ops:
                    pending_ops.pop(0)()
            prev = (atts, norm)
            if h == NH - 1:
                # queue out-projection of this ic for the next ic's slots
                pending_ops.extend(op_emitters(ic))
        # flush the final slot and remaining out-projections
        patts, pnorm = prev
        for e in patts:
            e()
        pnorm()
        for e in pending_ops:
            e()


def build(T=2048):
    nc = bass.Bass()
    with tile.TileContext(nc) as tc:
        with ExitStack() as ctx:
            _emit(nc, tc, ctx, T)
    return nc


def make_stair():
    j = np.arange(P)[:, None]
    u = np.arange(1024)[None, :]
    return (u >= j + 512).astype(ml_dtypes.bfloat16)


def make_in_maps(x, wq, bq, wk, bk, wv, bv, wo):
    bf = ml_dtypes.bfloat16
    f8 = ml_dtypes.float8_e4m3fn
    stair = make_stair()
    in_maps = []
    for c in range(8):
        b, g = c // 2, c % 2
        sl = slice(g * CL, (g + 1) * CL)
        xt = np.ascontiguousarray(x[b].T)
        in_maps.append(
            {
                "xT": xt.astype(bf),
                "xT8": xt.astype(f8),
                "wq8": np.ascontiguousarray(wq[:, sl] * 32.0).astype(f8),
                "wk8": np.ascontiguousarray(wk[:, sl] * 32.0).astype(f8),
                "wv": np.ascontiguousarray(wv[:, sl]).astype(bf),
                "wo": np.ascontiguousarray(wo[sl, :]).astype(bf),
                "bq": np.ascontiguousarray(bq[sl]),
                "bk": np.ascontiguousarray(bk[sl]),
                "bv": np.ascontiguousarray(bv[sl]).astype(bf),
                "stair": stair,
            }
        )
    return in_maps


_cache = {}


def _split_multi_waits(bir_json: bytes) -> bytes:
    """Split instructions carrying >1 sync waits into single-wait NoOp
    chains on the same engine queue.  The TPB instruction encoding has one
    wait slot; this walrus build refuses multi-wait instructions instead
    of splitting them itself."""
    import orjson

    m = orjson.loads(bir_json)
    n = 0
    for fn in m.get("functions", []):
        for blk in fn.get("blocks", []):
            out = []
            for inst in blk.get("instructions", []):
                si = inst.get("sync_info")
                waits = si.get("on_wait") if si else None
                if waits and len(waits) > 1:
                    for w in waits[:-1]:
                        n += 1
                        out.append(
                            {
                                "debug": inst.get("debug", {}),
                                "engine": inst["engine"],
                                "ins": [],
                                "outs": [],
                                "name": f"{inst['name']}_sw{n}",
                                "opcode": "NoOp",
                                "text_hint": "split_wait",
                                "sync_info": {"on_wait": [w], "on_update": []},
                            }
                        )
                    si["on_wait"] = [waits[-1]]
                out.append(inst)
            blk["instructions"] = out
    return orjson.dumps(m)


def _install_compile_patch():
    import concourse.bass_utils as bu

    if getattr(bu, "_split_waits_patched", False):
        return
    orig = bu.compile_bir_kernel

    def patched(bir_json, tmpdir, neff_name="file.neff"):
        return orig(_split_multi_waits(bir_json), tmpdir, neff_name)

    bu.compile_bir_kernel = patched
    bu._split_waits_patched = True
    try:
        import concourse.bass2jax as b2j

        b2j.compile_bir_kernel = patched
    except ImportError:
        pass


def kernel(x, wq, bq, wk, bk, wv, bv, wo, bo):
    from concourse.bass_utils import run_bass_kernel_spmd

    _install_compile_patch()

    x = np.asarray(x, np.float32)
    args = [np.asarray(a, np.float32) for a in (wq, bq, wk, bk, wv, bv, wo, bo)]
    wq, bq, wk, bk, wv, bv, wo, bo = args
    B, T, _ = x.shape

    if "nc" not in _cache:
        _cache["nc"] = build(T)
    nc = _cache["nc"]

    in_maps = make_in_maps(x, wq, bq, wk, bk, wv, bv, wo)
    res = run_bass_kernel_spmd(nc, in_maps, core_ids=list(range(8)))
    out = np.empty((B, T, C), np.float32)
    for b in range(B):
        out[b] = res.results[2 * b]["outp"] + res.results[2 * b + 1]["outp"] + bo
    return out


# revision 6
# speedup vs baseline: 1.3684x; 1.0313x over previous
"""Causal self-attention Trainium2 kernel.

Sharding: 8 cores = (4 batches) x (2 head-groups of 8 heads).
Each core: projections for its 512 channels, causal attention for its 8
heads over its batch, partial out-projection over its 512 channels.
Host: sums the two partials per batch and adds the output bias.

v2 design notes:
- q/k projections run as fp8 DoubleRow matmuls (2x128 contraction per
  pass -> half the PE row count); host pre-scales wq/wk by 32, the
  psum->sbuf conversion multiplies by 1/32 and adds the bias (on ACT,
  which idles during the projection phase). v stays bf16 (fp8 v noise
  breaks the 2e-2 budget).
- x arrives host-transposed (xT bf16 + xT8 fp8), no transpose DMAs.
- vx carries 64 ones-columns per head, so the attnV matmul broadcasts
  the softmax denominator onto psum partitions 64..127 for free
  (matmul cost depends only on the moving free size).
- normalization: DVE reciprocal [64,512] psum->bf16, then one
  tensor_mul (one-psum-operand rule) -> attT. No PE broadcast.
- emission is software-pipelined: scores of head-slot i interleave
  with attnV of slot i-1, and the out-projection of token-block ic
  interleaves into the head slots of ic+1, keeping PE fed while ACT
  works through the exps.
"""

from contextlib import ExitStack

import ml_dtypes
import numpy as np

import concourse.bass as bass
import concourse.mybir as mybir
import concourse.tile as tile

P = 128
C = 1024  # d_model
CL = 512  # local channels (8 heads * 64)
D = 64  # head dim
NH = 8  # local heads
FC = C // P  # 8 f-chunks
OC = CL // P  # 4 o-chunks
F32 = mybir.dt.float32
BF16 = mybir.dt.bfloat16
F8 = mybir.dt.float8e4
AF = mybir.ActivationFunctionType
DR = mybir.MatmulPerfMode.DoubleRow
GROUP = 3  # score jt-tiles per exp call


def _emit(nc, tc, ctx, T):
    NT = T // P  # 16 token chunks
    T4 = T // 512  # 4 ic-blocks

    xTd = nc.dram_tensor("xT", [C, T], BF16, kind="ExternalInput")
    xT8d = nc.dram_tensor("xT8", [C, T], F8, kind="ExternalInput")
    wq8d = nc.dram_tensor("wq8", [C, CL], F8, kind="ExternalInput")
    wk8d = nc.dram_tensor("wk8", [C, CL], F8, kind="ExternalInput")
    wvd = nc.dram_tensor("wv", [C, CL], BF16, kind="ExternalInput")
    wod = nc.dram_tensor("wo", [CL, C], BF16, kind="ExternalInput")
    bqd = nc.dram_tensor("bq", [CL], F32, kind="ExternalInput")
    bkd = nc.dram_tensor("bk", [CL], F32, kind="ExternalInput")
    bvd = nc.dram_tensor("bv", [CL], BF16, kind="ExternalInput")
    staird = nc.dram_tensor("stair", [P, 1024], BF16, kind="ExternalInput")
    identd = nc.dram_tensor("ident", [P, P], BF16, kind="ExternalInput")
    outp = nc.dram_tensor("outp", [T, C], F32, kind="ExternalOutput")

    const = ctx.enter_context(tc.tile_pool(name="const", bufs=1))
    ones1 = const.tile([1, P], BF16)
    nc.gpsimd.memset(ones1[:], 1.0)
    bq_sb = const.tile([P, OC], F32)
    nc.sync.dma_start(bq_sb[:], bqd.rearrange("(oc p) -> p oc", p=P))
    bk_sb = const.tile([P, OC], F32)
    nc.sync.dma_start(bk_sb[:], bkd.rearrange("(oc p) -> p oc", p=P))
    bv_sb = const.tile([1, CL], BF16)
    nc.sync.dma_start(bv_sb[:], bvd.rearrange("(a c) -> a c", a=1))
    stair_sb = const.tile([P, 1024], BF16)
    nc.sync.dma_start(stair_sb[:], staird[:])
    ident_sb = const.tile([P, P], BF16)
    nc.sync.dma_start(ident_sb[:], identd[:])

    main = ctx.enter_context(tc.tile_pool(name="main", bufs=1))
    qT = main.tile([P, OC, T], BF16)
    kT = main.tile([P, OC, T], BF16)
    vx = main.tile([P, NT, NH, P], BF16)
    nc.gpsimd.memset(vx[:, :, :, D:P], 1.0)
    attT = main.tile([P, OC, T], BF16)
    wo_sb = main.tile([P, OC, C], BF16)
    nc.sync.dma_start(wo_sb[:], wod.rearrange("(oc p) c -> p oc c", p=P))

    # ---------------- projections ----------------
    with (
        tc.tile_pool(name="wpool", bufs=1) as wpool,
        tc.tile_pool(name="xpool", bufs=1) as xpool,
        tc.tile_pool(name="pj_ps", bufs=4, space="PSUM") as pj_ps,
    ):
        xT8 = xpool.tile([P, FC, T], F8)
        xT8r = xT8d.rearrange("(fc p) t -> p fc t", p=P)
        for fc in range(FC):
            nc.sync.dma_start(xT8[:, fc, :], xT8r[:, fc, :])
        wq8_sb = wpool.tile([P, FC, CL], F8)
        nc.sync.dma_start(wq8_sb[:], wq8d.rearrange("(fc p) o -> p fc o", p=P))
        wk8_sb = wpool.tile([P, FC, CL], F8)
        nc.sync.dma_start(wk8_sb[:], wk8d.rearrange("(fc p) o -> p fc o", p=P))
        xT = xpool.tile([P, FC, T], BF16)
        xTr = xTd.rearrange("(fc p) t -> p fc t", p=P)
        for fc in range(FC):
            nc.sync.dma_start(xT[:, fc, :], xTr[:, fc, :])
        wv_sb = wpool.tile([P, FC, CL], BF16)
        nc.sync.dma_start(wv_sb[:], wvd.rearrange("(fc p) o -> p fc o", p=P))

        # q/k projections: fp8 DoubleRow, contraction 4 passes of 2x128
        for oc in range(OC):
            for w8, b_sb, dT in ((wq8_sb, bq_sb, qT), (wk8_sb, bk_sb, kT)):
                for tt in range(T4):
                    ps = pj_ps.tile([P, 512], F32, tag="pj", name=f"pj{oc}_{tt}_{dT is qT}")
                    for g in range(4):
                        nc.tensor.matmul(
                            ps[:],
                            w8[:, 2 * g : 2 * g + 2, oc * P : (oc + 1) * P],
                            xT8[:, 2 * g : 2 * g + 2, tt * 512 : (tt + 1) * 512],
                            start=(g == 0),
                            stop=(g == 3),
                            perf_mode=DR,
                        )
                    nc.scalar.activation(
                        dT[:, oc, tt * 512 : (tt + 1) * 512],
                        ps[:],
                        AF.Identity,
                        bias=b_sb[:, oc : oc + 1],
                        scale=0.03125,
                    )
        # v projection: bf16, xT chunks stationary, wv moving
        for s in range(NT):
            ps = pj_ps.tile([P, 512], F32, tag="pj", name=f"pjv{s}")
            for fc in range(FC):
                nc.tensor.matmul(
                    ps[:],
                    xT[:, fc, s * P : (s + 1) * P],
                    wv_sb[:, fc, :],
                    start=(fc == 0),
                    stop=False,
                )
            nc.tensor.matmul(ps[:], ones1[:], bv_sb[:], start=False, stop=True)
            nc.scalar.copy(
                vx[:, s, :, 0:D],
                ps[:].rearrange("p (h d) -> p h d", d=D),
            )

    # ---------------- attention + out-projection ----------------
    with (
        tc.tile_pool(name="ex_pool", bufs=7) as ex_pool,
        tc.tile_pool(name="nrm", bufs=2) as nrm_pool,
        tc.tile_pool(name="ob_pool", bufs=4) as ob_pool,
        tc.tile_pool(name="sc_ps", bufs=2, space="PSUM") as sc_ps_pool,
        tc.tile_pool(name="at_ps", bufs=2, space="PSUM") as at_ps_pool,
    ):
        # slot list: (ic, h); slot i's scores interleave with slot i-1's attnV
        slots = [(ic, h) for ic in reversed(range(T4)) for h in range(NH)]

        def sc_emitters(ic, h, sc_tiles):
            """Score-group closures for (ic, h): matmul njt k-tiles in
            GROUP-sized chunks + exp on ACT; fill sc_tiles with
            (sc, ex, grp) for the attnV closures."""
            oc, hh = h // 2, h % 2
            base = hh * D
            njt = ic * 4 + 4
            outs = []
            for g0 in range(0, njt, GROUP):
                grp = list(range(g0, min(g0 + GROUP, njt)))

                def emit(grp=grp, oc=oc, base=base, ic=ic, h=h):
                    n = len(grp)
                    sc = sc_ps_pool.tile(
                        [P, GROUP, 512], F32, tag="sc", name=f"sc{ic}_{h}_{grp[0]}"
                    )
                    for si, jt in enumerate(grp):
                        d = jt - ic * 4
                        nc.tensor.matmul(
                            sc[:, si, :],
                            kT[base : base + D, oc, jt * P : (jt + 1) * P],
                            qT[base : base + D, oc, ic * 512 : (ic + 1) * 512],
                            start=True,
                            stop=(d < 0),
                        )
                        if d >= 0:
                            w = (d + 1) * P
                            nc.tensor.matmul(
                                sc[:, si, 0:w],
                                ident_sb[:],
                                stair_sb[:, 512 - d * P : 512 - d * P + w],
                                start=False,
                                stop=True,
                                skip_group_check=True,
                            )
                    ex = ex_pool.tile(
                        [P, GROUP, 512], BF16, tag="ex", name=f"ex{ic}_{h}_{grp[0]}"
                    )
                    nc.scalar.activation(
                        ex[:, 0:n, :], sc[:, 0:n, :], AF.Exp, scale=0.125
                    )
                    sc_tiles.append((ex, grp))

                outs.append(emit)
            return outs

        def att_emitters(ic, h, sc_tiles, at_box):
            """attnV-group closures for (ic, h): stair-mask diag tiles on
            DVE, then accumulate into the at psum tile."""
            njt = ic * 4 + 4
            outs = []
            ngrp = (njt + GROUP - 1) // GROUP

            for gi in range(ngrp):

                def emit(gi=gi, ic=ic, h=h):
                    ex, grp = sc_tiles[gi]
                    if gi == 0:
                        at_box.append(
                            at_ps_pool.tile([P, 512], F32, tag="at", name=f"at{ic}_{h}")
                        )
                    at = at_box[0]
                    njt_l = ic * 4 + 4
                    for si, jt in enumerate(grp):
                        nc.tensor.matmul(
                            at[:],
                            vx[:, jt, h, :],
                            ex[:, si, :],
                            start=(jt == 0),
                            stop=(jt == njt_l - 1),
                        )

                outs.append(emit)
            return outs

        def norm_emit(ic, h, at_box):
            oc, hh = h // 2, h % 2
            base = hh * D
            at = at_box[0]
            rc = nrm_pool.tile([D, 512], BF16, tag="rc", name=f"rc{ic}_{h}")
            with nc.allow_low_precision(reason="softmax recip bcast"):
                nc.vector.reciprocal(rc[:], at[D:P, :])
            nc.vector.tensor_mul(
                attT[base : base + D, oc, ic * 512 : (ic + 1) * 512],
                at[0:D, :],
                rc[:],
            )

        def op_emitters(ic):
            """Out-projection chains for token block ic: 4 token chunks x
            2 channel halves, each a 4-matmul chain + copy + store."""
            outs = []
            for s in range(4):
                for ch in range(2):

                    def emit(s=s, ch=ch, ic=ic):
                        s16 = ic * 4 + s
                        ps = at_ps_pool.tile(
                            [P, 512], F32, tag="at", name=f"op{ic}_{s}_{ch}"
                        )
                        for oc in range(OC):
                            nc.tensor.matmul(
                                ps[:],
                                attT[:, oc, s16 * P : (s16 + 1) * P],
                                wo_sb[:, oc, ch * 512 : (ch + 1) * 512],
                                start=(oc == 0),
                                stop=(oc == OC - 1),
                            )
                        ob = ob_pool.tile([P, 512], F32, tag="ob", name=f"ob{ic}_{s}_{ch}")
                        nc.vector.tensor_copy(ob[:], ps[:])
                        nc.gpsimd.dma_start(
                            outp[s16 * P : (s16 + 1) * P, ch * 512 : (ch + 1) * 512],
                            ob[:],
                        )

                    outs.append(emit)
            return outs

        prev = None  # (att_closures, norm_closure) of slot i-1
        pending_ops = []
        for ic, h in slots:
            sc_tiles = []
            at_box = []
            scs = sc_emitters(ic, h, sc_tiles)
            atts = att_emitters(ic, h, sc_tiles, at_box)
            norm = lambda ic=ic, h=h, ab=at_box: norm_emit(ic, h, ab)

            # interleave: this slot's scores with previous slot's attnV
            if prev is None:
                for e in scs:
                    e()
            else:
                patts, pnorm = prev
                # lead with two score groups, then alternate
                na, nb = len(scs), len(patts)
                ai = bi = 0
                while ai < na or bi < nb:
                    if ai < na:
                        e = scs[ai]
                        ai += 1
                        e()
                    if ai < na and ai <= bi + 1:
                        e = scs[ai]
                        ai += 1
                        e()
                    if bi < nb:
                        e = patts[bi]
                        bi += 1
                        e()
                pnorm()
                if pending_ops:
                    pending_ops.pop(0)()
            prev = (atts, norm)
            if h == NH - 1:
                # queue out-projection of this ic for the next ic's slots
                pending_ops.extend(op_emitters(ic))
        # flush the final slot and remaining out-projections
        patts, pnorm = prev
        for e in patts:
            e()
        pnorm()
        for e in pending_ops:
            e()


def build(T=2048):
    nc = bass.Bass()
    with tile.TileContext(nc) as tc:
        with ExitStack() as ctx:
            _emit(nc, tc, ctx, T)
    return nc


def make_stair():
    # additive causal mask: -1024 where the key is ahead of the query
    j = np.arange(P)[:, None]
    u = np.arange(1024)[None, :]
    return np.where(u < j + 512, -1024.0, 0.0).astype(ml_dtypes.bfloat16)


def make_in_maps(x, wq, bq, wk, bk, wv, bv, wo):
    bf = ml_dtypes.bfloat16
    f8 = ml_dtypes.float8_e4m3fn
    stair = make_stair()
    in_maps = []
    for c in range(8):
        b, g = c // 2, c % 2
        sl = slice(g * CL, (g + 1) * CL)
        xt = np.ascontiguousarray(x[b].T)
        in_maps.append(
            {
                "xT": xt.astype(bf),
                "xT8": xt.astype(f8),
                "wq8": np.ascontiguousarray(wq[:, sl] * 32.0).astype(f8),
                "wk8": np.ascontiguousarray(wk[:, sl] * 32.0).astype(f8),
                "wv": np.ascontiguousarray(wv[:, sl]).astype(bf),
                "wo": np.ascontiguousarray(wo[sl, :]).astype(bf),
                "bq": np.ascontiguousarray(bq[sl]),
                "bk": np.ascontiguousarray(bk[sl]),
                "bv": np.ascontiguousarray(bv[sl]).astype(bf),
                "stair": stair,
                "ident": np.eye(P, dtype=ml_dtypes.bfloat16),
            }
        )
    return in_maps


_cache = {}


def _split_multi_waits(bir_json: bytes) -> bytes:
    """Split instructions carrying >1 sync waits into single-wait NoOp
    chains on the same engine queue.  The TPB instruction encoding has one
    wait slot; this walrus build refuses multi-wait instructions instead
    of splitting them itself."""
    import orjson

    m = orjson.loads(bir_json)
    n = 0
    for fn in m.get("functions", []):
        for blk in fn.get("blocks", []):
            out = []
            for inst in blk.get("instructions", []):
                si = inst.get("sync_info")
                waits = si.get("on_wait") if si else None
                if waits and len(waits) > 1:
                    for w in waits[:-1]:
                        n += 1
                        out.append(
                            {
                                "debug": inst.get("debug", {}),
                                "engine": inst["engine"],
                                "ins": [],
                                "outs": [],
                                "name": f"{inst['name']}_sw{n}",
                                "opcode": "NoOp",
                                "text_hint": "split_wait",
                                "sync_info": {"on_wait": [w], "on_update": []},
                            }
                        )
                    si["on_wait"] = [waits[-1]]
                out.append(inst)
            blk["instructions"] = out
    return orjson.dumps(m)


def _install_compile_patch():
    import concourse.bass_utils as bu

    if getattr(bu, "_split_waits_patched", False):
        return
    orig = bu.compile_bir_kernel

    def patched(bir_json, tmpdir, neff_name="file.neff"):
        return orig(_split_multi_waits(bir_json), tmpdir, neff_name)

    bu.compile_bir_kernel = patched
    bu._split_waits_patched = True
    try:
        import concourse.bass2jax as b2j

        b2j.compile_bir_kernel = patched
    except ImportError:
        pass


def kernel(x, wq, bq, wk, bk, wv, bv, wo, bo):
    from concourse.bass_utils import run_bass_kernel_spmd

    _install_compile_patch()

    x = np.asarray(x, np.float32)
    args = [np.asarray(a, np.float32) for a in (wq, bq, wk, bk, wv, bv, wo, bo)]
    wq, bq, wk, bk, wv, bv, wo, bo = args
    B, T, _ = x.shape

    if "nc" not in _cache:
        _cache["nc"] = build(T)
    nc = _cache["nc"]

    in_maps = make_in_maps(x, wq, bq, wk, bk, wv, bv, wo)
    res = run_bass_kernel_spmd(nc, in_maps, core_ids=list(range(8)))
    out = np.empty((B, T, C), np.float32)
    for b in range(B):
        out[b] = res.results[2 * b]["outp"] + res.results[2 * b + 1]["outp"] + bo
    return out
